# revision 8
# baseline (speedup 1.0000x reference)
"""Cross-attention kernel for Trainium2 (8 NeuronCores, SPMD data-parallel).

Problem: B=4, C=128, 64x64 spatial (N=4096 tokens), 4 heads of dim 32.
  q = Wq @ query; k = Wk @ key; v = Wv @ key   (1x1 convs == channel matmuls)
  out = softmax(q^T k / sqrt(32)) @ v          (per batch*head)

Sharding: 16 (batch, head) jobs -> 2 per core. Core i handles batch i//2,
heads {2*(i%2), 2*(i%2)+1} i.e. output channels [64*(i%2), 64*(i%2)+64).

Structure (per core):
  - Projections fold into host-side input prep (tiny vs. the N^2 attention).
    Per head the score matmul is rank-32: s = (Wq_h q)^T (Wk_h k). The fp8
    quantization error is cancelled to first order by a hi/lo split packed
    into the contraction dim (free under the PE cost model): 96 rows =
    {q_hi.k_hi, q_lo.k_hi, q_hi.k_lo} shipped as DoubleRow pairs [48, 2, N],
    so one fp8 DoubleRow matmul yields scoresT chunk [nk=128, nq=512] at
    half bf16 cost with ~bf16 accuracy.
  - exp: the N^2 score surface must drain PSUM->SBUF through ACT/DVE at
    1 elem/cycle/lane (GPSIMD and DMA cannot touch PSUM) - that drain is the
    kernel floor. Tiles are [128, 1024] (2 chunks, 2 PSUM banks) x 3 bufs so
    the exp(t) -> QK(t+3) -> exp(t+3) WAR chain amortizes over 3 tiles;
    engines are assigned by a greedy balance of modeled op costs
    (ACT ~1038ns vs DVE ~1192ns per tile).
  - PV flipped: ctx[nq=128, 33] += probsT_chunk(lhsT, stationary) @ v_aug
    (moving: 32 v columns + a ones column -> softmax denominator), bf16.
  - host: normalize by the denominator column and transpose to [C, N].
"""

import functools
import math

import numpy as np

NCORES = 8
B, C, HS, WS = 4, 128, 64, 64
N = HS * WS  # 4096 tokens
NUM_HEADS = 4
DH = 32  # head dim
HPC = 2  # heads per core

NQB = 512  # nq per QK matmul (one PSUM bank of f32)
NKC = 128  # nk chunk (PV contraction tile)
N_BLOCKS = N // NQB  # 8
N_CHUNKS = N // NKC  # 32
VTW = 33  # v^T tile width: 32 v cols + 1 ones col (denominator)
CPT = 2  # chunks per exp tile
TPB = N_CHUNKS // CPT  # 16 exp tiles per (h, block)

# host-side scale on the q projection (keeps fp8 operands ~unit variance)
BETA = 2.0
# device scores s_dev = BETA * (q.k); log2-domain y = s_dev * Y_PER_S
Y_PER_S = math.log2(math.e) / (BETA * math.sqrt(DH))
# Schraudolph exp2 in bf16: i16 = cvt(y*128 + (16256 - C)); bits = bf16 ~ 2^y
EXP2_A = 128.0 * Y_PER_S
EXP2_B = 16256.0 - 5.25
# ACT exact exp: exp(ACT_SCALE * s_dev) = 2^y
ACT_SCALE = math.log(2.0) * Y_PER_S

# modeled per-op engine costs (ns) used for the static plan
_COST = {"scA": 1038.0, "scD1": 1192.0, "scD2": 658.0}
_COST_A_OB = 295.0
_COST_D_OB = 262.0
# block mixes: (a A-tiles, d D1/D2 pairs): 2a + 3d = 32 chunks.
# 9 blocks of (10, 4) + 7 of (7, 6) balances ACT/DVE at ~144.3us each.
_BLOCK_MIX = [(10, 4) if i % 2 == 0 or i == 15 else (7, 6) for i in range(16)]


def _exp_plan():
    """Per-block tile schedule. Each engine owns private sc pools so its WAR
    turnaround hides under same-engine serial time: ACT uses scA (2 bufs of
    [128,1024]); DVE alternates scD1 ([128,1024], 1 buf) and scD2
    ([128,512], 1 buf) to amortize per-op overhead while keeping 2 D-tiles
    in flight. Tiles interleave by modeled virtual finish time. Returns
    (tiles, ob_eng): tiles[block] = list of (pool, nchunks)."""
    tiles = []
    ob_eng = []
    ta = td = 0.0
    for a_tiles, d_pairs in _BLOCK_MIX:
        arow = [("scA", CPT)] * a_tiles
        drow = [("scD1", 2), ("scD2", 1)] * d_pairs
        row = []
        ia = idd = 0
        va = vd = 0.0
        while ia < len(arow) or idd < len(drow):
            ca = va + _COST[arow[ia][0]] if ia < len(arow) else None
            cd = vd + _COST[drow[idd][0]] if idd < len(drow) else None
            if cd is None or (ca is not None and ca <= cd):
                row.append(arow[ia])
                va = ca
                ia += 1
            else:
                row.append(drow[idd])
                vd = cd
                idd += 1
        ta += va
        td += vd
        if ta + _COST_A_OB <= td + _COST_D_OB:
            ob_eng.append("A")
            ta += _COST_A_OB
        else:
            ob_eng.append("D")
            td += _COST_D_OB
        tiles.append(row)
    return tiles, ob_eng


def _f32(x):
    return np.ascontiguousarray(np.asarray(x, dtype=np.float32))


def _bf16(x):
    import ml_dtypes

    return np.ascontiguousarray(
        np.asarray(x, dtype=np.float32).astype(ml_dtypes.bfloat16)
    )


def _fp8(x):
    import ml_dtypes

    return np.ascontiguousarray(
        np.asarray(x, dtype=np.float32).astype(ml_dtypes.float8_e4m3)
    )


@functools.lru_cache(maxsize=1)
def _build_program():
    from contextlib import ExitStack

    import concourse.tile as tile
    from concourse import bacc, mybir
    from concourse.bass import ts

    f32 = mybir.dt.float32
    bf16 = mybir.dt.bfloat16
    i16 = mybir.dt.int16
    fp8 = mybir.dt.float8e4
    AF = mybir.ActivationFunctionType
    ALU = mybir.AluOpType
    PM = mybir.MatmulPerfMode

    nc = bacc.Bacc(
        "TRN2",
        target_bir_lowering=False,
        debug=False,
        enable_asserts=False,
        num_devices=NCORES,
    )

    # hi/lo fp8 pair layout [48, 2*N]: element (p, j*N + n) = G[j*48 + p, n]
    # where G is the 96-row stack (see _shard_inputs).
    kqi = [
        (
            nc.dram_tensor(f"k8_{h}", [48, 2 * N], fp8, kind="ExternalInput").ap(),
            nc.dram_tensor(f"q8_{h}", [48, 2 * N], fp8, kind="ExternalInput").ap(),
        )
        for h in range(HPC)
    ]
    vti = [
        nc.dram_tensor(f"vt{h}", [128, VTW * N_CHUNKS], bf16, kind="ExternalInput").ap()
        for h in range(HPC)
    ]

    # per (h, nq-block): ctx rows [nq=128 x 4 j-tiles], cols 32 ctx + 1 den
    out_ctx = nc.dram_tensor(
        "out_ctx", [HPC * N_BLOCKS, 128, 4 * VTW], f32, kind="ExternalOutput"
    ).ap()

    tiles_plan, ob_eng = _exp_plan()

    with tile.TileContext(nc) as tc, ExitStack() as ctx:
        persist = ctx.enter_context(tc.tile_pool(name="persist", bufs=1))

        k8 = [persist.tile([48, 2 * N], fp8, name=f"k8s{h}") for h in range(HPC)]
        q8 = [persist.tile([48, 2 * N], fp8, name=f"q8s{h}") for h in range(HPC)]
        vt = [
            persist.tile([128, VTW * N_CHUNKS], bf16, name=f"vts{h}")
            for h in range(HPC)
        ]
        k8v = [k8[h].rearrange("p (k n) -> p k n", k=2) for h in range(HPC)]
        q8v = [q8[h].rearrange("p (k n) -> p k n", k=2) for h in range(HPC)]
        kqiv = [
            (t0.rearrange("p (k n) -> p k n", k=2), t1.rearrange("p (k n) -> p k n", k=2))
            for t0, t1 in kqi
        ]
        # load order: the first tiles of head 0 gate the pipeline start
        nc.sync.dma_start(out=q8v[0][:, :, 0:NQB], in_=kqiv[0][1][:, :, 0:NQB])
        nc.sync.dma_start(out=k8v[0][:, :, 0:1024], in_=kqiv[0][0][:, :, 0:1024])
        nc.sync.dma_start(out=vt[0], in_=vti[0])
        nc.sync.dma_start(out=k8v[0][:, :, 1024:N], in_=kqiv[0][0][:, :, 1024:N])
        nc.sync.dma_start(out=q8v[0][:, :, NQB:N], in_=kqiv[0][1][:, :, NQB:N])
        nc.sync.dma_start(out=k8[1], in_=kqi[1][0])
        nc.sync.dma_start(out=q8[1], in_=kqi[1][1])
        nc.sync.dma_start(out=vt[1], in_=vti[1])

        pools = {
            "scA": ctx.enter_context(tc.tile_pool(name="scA", bufs=2, space="PSUM")),
            "scD1": ctx.enter_context(tc.tile_pool(name="scD1", bufs=1, space="PSUM")),
            "scD2": ctx.enter_context(tc.tile_pool(name="scD2", bufs=1, space="PSUM")),
        }
        ctx_pool = ctx.enter_context(tc.tile_pool(name="ctxp", bufs=1, space="PSUM"))
        ex_pool = ctx.enter_context(tc.tile_pool(name="ex", bufs=8))
        ob_pool = ctx.enter_context(tc.tile_pool(name="obp", bufs=4))

        # ---- attention ----
        for h in range(HPC):
            for b in range(N_BLOCKS):
                blk = h * N_BLOCKS + b
                ctx_ps = ctx_pool.tile([128, 4 * VTW], f32, name="ctx_ps")
                cbase = 0
                for pool, nch in tiles_plan[blk]:
                    eng = "A" if pool == "scA" else "D"
                    w = nch * NQB
                    sc = pools[pool].tile([128, w], f32, name=pool, tag=pool)
                    with tc.high_priority(offset=320):
                        for u in range(nch):
                            c = cbase + u
                            nc.tensor.matmul(
                                out=sc[:, ts(u, NQB)],
                                lhsT=k8v[h][:, :, ts(c, NKC)],
                                rhs=q8v[h][:, :, ts(b, NQB)],
                                start=True,
                                stop=True,
                                perf_mode=PM.DoubleRow,
                            )
                    ex = ex_pool.tile([128, w], bf16, name="ex")
                    if eng == "A":
                        nc.scalar.activation(ex, sc, AF.Exp, scale=ACT_SCALE)
                    else:
                        nc.vector.tensor_scalar(
                            ex.bitcast(i16), sc, EXP2_A, EXP2_B,
                            op0=ALU.mult, op1=ALU.add,
                        )
                    for u in range(nch):
                        c = cbase + u
                        for j in range(4):
                            # NOTE: start=True clears has_written BANK-wide,
                            # so only the tile's very first matmul may set it
                            # (the bit-clear makes every region's first write
                            # an overwrite, later writes accumulate).
                            nc.tensor.matmul(
                                out=ctx_ps[:, ts(j, VTW)],
                                lhsT=ex[:, NQB * u + NKC * j : NQB * u + NKC * (j + 1)],
                                rhs=vt[h][:, ts(c, VTW)],
                                start=(c == 0 and j == 0),
                                stop=(c == N_CHUNKS - 1 and j == 3),
                                skip_group_check=True,
                            )
                    cbase += nch
                ob = ob_pool.tile([128, 4 * VTW], f32, name="ob")
                if ob_eng[blk] == "A":
                    nc.scalar.copy(out=ob, in_=ctx_ps)
                else:
                    nc.vector.tensor_copy(out=ob, in_=ctx_ps)
                nc.sync.dma_start(out=out_ctx[blk], in_=ob)

    nc.compile()
    return nc


def _shard_inputs(query, key, Wq, Wk, Wv):
    import ml_dtypes

    query = _f32(query).reshape(B, C, N)
    key = _f32(key).reshape(B, C, N)
    Wq, Wk, Wv = _f32(Wq), _f32(Wk), _f32(Wv)

    def pack96(G):
        # G [96, N] fp8-valued f32 -> [48, 2*N] with (p, j*N+n) = G[j*48+p, n]
        return np.ascontiguousarray(
            G.reshape(2, 48, N).transpose(1, 0, 2).reshape(48, 2 * N)
        )

    in_maps = []
    for core in range(NCORES):
        b, half = core // 2, core % 2
        im = {}
        for hl in range(HPC):
            ch0 = 64 * half + 32 * hl
            wq_h = Wq[ch0 : ch0 + 32, :]  # [32, 128]
            wk_h = Wk[ch0 : ch0 + 32, :]
            wv_h = Wv[ch0 : ch0 + 32, :]
            qs = BETA * (wq_h @ query[b])  # [32, N]
            ks = wk_h @ key[b]  # [32, N]
            q_hi = _fp8(qs)
            k_hi = _fp8(ks)
            q_lo = _fp8(qs - np.asarray(q_hi, np.float32))
            k_lo = _fp8(ks - np.asarray(k_hi, np.float32))
            # contraction rows: q_hi.k_hi + q_lo.k_hi + q_hi.k_lo
            Gq = np.concatenate([q_hi, q_lo, q_hi]).astype(ml_dtypes.float8_e4m3)
            Gk = np.concatenate([k_hi, k_hi, k_lo]).astype(ml_dtypes.float8_e4m3)
            im[f"q8_{hl}"] = pack96(Gq)
            im[f"k8_{hl}"] = pack96(Gk)
            vta = np.ones((N, VTW), np.float32)
            vta[:, :32] = key[b].T @ wv_h.T
            im[f"vt{hl}"] = _bf16(
                vta.reshape(N_CHUNKS, NKC, VTW)
                .transpose(1, 0, 2)
                .reshape(NKC, N_CHUNKS * VTW)
            )
        in_maps.append(im)
    return in_maps


def _run(in_maps, trace=False):
    from concourse import bass_utils

    nc = _build_program()
    return bass_utils.run_bass_kernel_spmd(
        nc, in_maps, core_ids=list(range(NCORES)), trace=trace
    )


def _assemble(results):
    out = np.empty((B, C, N), np.float32)
    for core in range(NCORES):
        b, half = core // 2, core % 2
        r = results[core]
        t = np.asarray(r["out_ctx"], np.float32)  # [16, 128, 132]
        t = t.reshape(HPC, N_BLOCKS, 128, 4, VTW)
        ctx = t[..., :32]  # [h, b8, p, j, d]
        den = t[..., 32]  # [h, b8, p, j]
        # nq index = b8*512 + j*128 + p -> order (b8, j, p)
        ctx = np.transpose(ctx, (0, 1, 3, 2, 4)).reshape(HPC, N, 32)
        den = np.transpose(den, (0, 1, 3, 2)).reshape(HPC, N)
        for hl in range(HPC):
            ch0 = 64 * half + 32 * hl
            out[b, ch0 : ch0 + 32, :] = (ctx[hl] / den[hl][:, None]).T
    return out.reshape(B, C, HS, WS)


def kernel(query, key, Wq, Wk, Wv):
    in_maps = _shard_inputs(query, key, Wq, Wk, Wv)
    res = _run(in_maps)
    return _assemble(res.results)


# revision 12
# speedup vs baseline: 1.0615x; 1.0615x over previous
"""Cross-attention kernel for Trainium2 (8 NeuronCores, SPMD data-parallel).

Problem: B=4, C=128, 64x64 spatial (N=4096 tokens), 4 heads of dim 32.
  q = Wq @ query; k = Wk @ key; v = Wv @ key   (1x1 convs == channel matmuls)
  out = softmax(q^T k / sqrt(32)) @ v          (per batch*head)

Sharding: 16 (batch, head) jobs -> 2 per core. Core i handles batch i//2,
heads {2*(i%2), 2*(i%2)+1} i.e. output channels [64*(i%2), 64*(i%2)+64).

Structure (per core):
  - Projections fold into host-side input prep (tiny vs. the N^2 attention).
    Per head the score matmul is rank-32: s = (Wq_h q)^T (Wk_h k). The fp8
    quantization error is cancelled to first order by a hi/lo split packed
    into the contraction dim (free under the PE cost model): 96 rows =
    {q_hi.k_hi, q_lo.k_hi, q_hi.k_lo} shipped as DoubleRow pairs [48, 2, N],
    so one fp8 DoubleRow matmul yields scoresT chunk [nk=128, nq=512] at
    half bf16 cost with ~bf16 accuracy.
  - exp: the N^2 score surface must drain PSUM->SBUF through ACT/DVE at
    1 elem/cycle/lane (GPSIMD and DMA cannot touch PSUM) - that drain is the
    kernel floor. Tiles are [128, 1024] (2 chunks, 2 PSUM banks) x 3 bufs so
    the exp(t) -> QK(t+3) -> exp(t+3) WAR chain amortizes over 3 tiles;
    engines are assigned by a greedy balance of modeled op costs
    (ACT ~1038ns vs DVE ~1192ns per tile).
  - PV flipped: ctx[nq=128, 33] += probsT_chunk(lhsT, stationary) @ v_aug
    (moving: 32 v columns + a ones column -> softmax denominator), bf16.
  - host: normalize by the denominator column and transpose to [C, N].
"""

import functools
import math

import numpy as np

NCORES = 8
B, C, HS, WS = 4, 128, 64, 64
N = HS * WS  # 4096 tokens
NUM_HEADS = 4
DH = 32  # head dim
HPC = 2  # heads per core

NQB = 512  # nq per QK matmul (one PSUM bank of f32)
NKC = 128  # nk chunk (PV contraction tile)
N_BLOCKS = N // NQB  # 8
N_CHUNKS = N // NKC  # 32
VTW = 33  # v^T tile width: 32 v cols + 1 ones col (denominator)
CPT = 2  # chunks per exp tile
TPB = N_CHUNKS // CPT  # 16 exp tiles per (h, block)

# host-side scale on the q projection (keeps fp8 operands ~unit variance)
BETA = 2.0
# device scores s_dev = BETA * (q.k); log2-domain y = s_dev * Y_PER_S
Y_PER_S = math.log2(math.e) / (BETA * math.sqrt(DH))
# Schraudolph exp2 in bf16: i16 = cvt(y*128 + (16256 - C)); bits = bf16 ~ 2^y
EXP2_A = 128.0 * Y_PER_S
EXP2_B = 16256.0 - 5.25
# ACT exact exp: exp(ACT_SCALE * s_dev) = 2^y
ACT_SCALE = math.log(2.0) * Y_PER_S

# modeled per-op engine costs (ns) used for the static plan
_COST_A_TILE = 1038.0  # ACT [128,1024] from PSUM
_COST_D_TILE = 658.0  # DVE [128,512] from PSUM
_COST_A_OB = 295.0
_COST_D_OB = 262.0
A_TILES = 9  # ACT tiles (2 chunks each) per block
D_TILES = N_CHUNKS - 2 * A_TILES  # DVE tiles (1 chunk each) per block
N_BLK = HPC * N_BLOCKS


def _exp_plan():
    """Per-block tile schedule: each engine owns a private double-buffered
    sc pool so its WAR turnaround hides under same-engine serial time.
    Interleave A (2-chunk) and D (1-chunk) tiles by modeled virtual finish
    time. The final block front-loads A tiles (and gets ob on ACT) so both
    engines drain together at the end. Returns (tiles, ob_eng):
    tiles[block] = list of (("A"|"D"), nchunks)."""
    tiles = []
    ob_eng = []
    ta = td = 0.0
    for blk in range(N_BLK):
        row = []
        na = nd = 0
        va = vd = 0.0
        while na < A_TILES or nd < D_TILES:
            pick_a = na < A_TILES and (
                nd >= D_TILES
                or (blk == N_BLK - 1)  # front-load A in the last block
                or va + _COST_A_TILE <= vd + _COST_D_TILE
            )
            if pick_a:
                row.append(("A", CPT))
                va += _COST_A_TILE
                na += 1
            else:
                row.append(("D", 1))
                vd += _COST_D_TILE
                nd += 1
        ta += va
        td += vd
        if blk == N_BLK - 1 or ta + _COST_A_OB <= td + _COST_D_OB:
            ob_eng.append("A")
            ta += _COST_A_OB
        else:
            ob_eng.append("D")
            td += _COST_D_OB
        tiles.append(row)
    return tiles, ob_eng


def _f32(x):
    return np.ascontiguousarray(np.asarray(x, dtype=np.float32))


def _bf16(x):
    import ml_dtypes

    return np.ascontiguousarray(
        np.asarray(x, dtype=np.float32).astype(ml_dtypes.bfloat16)
    )


def _fp8(x):
    import ml_dtypes

    return np.ascontiguousarray(
        np.asarray(x, dtype=np.float32).astype(ml_dtypes.float8_e4m3)
    )


@functools.lru_cache(maxsize=1)
def _build_program():
    from contextlib import ExitStack

    import concourse.tile as tile
    from concourse import bacc, mybir
    from concourse.bass import ts

    f32 = mybir.dt.float32
    bf16 = mybir.dt.bfloat16
    i16 = mybir.dt.int16
    fp8 = mybir.dt.float8e4
    AF = mybir.ActivationFunctionType
    ALU = mybir.AluOpType
    PM = mybir.MatmulPerfMode

    nc = bacc.Bacc(
        "TRN2",
        target_bir_lowering=False,
        debug=False,
        enable_asserts=False,
        num_devices=NCORES,
    )

    # hi/lo fp8 pair layout [48, 2*N]: element (p, j*N + n) = G[j*48 + p, n]
    # where G is the 96-row stack (see _shard_inputs).
    kqi = [
        (
            nc.dram_tensor(f"k8_{h}", [48, 2 * N], fp8, kind="ExternalInput").ap(),
            nc.dram_tensor(f"q8_{h}", [48, 2 * N], fp8, kind="ExternalInput").ap(),
        )
        for h in range(HPC)
    ]
    vti = [
        nc.dram_tensor(f"vt{h}", [128, VTW * N_CHUNKS], bf16, kind="ExternalInput").ap()
        for h in range(HPC)
    ]

    # per (h, nq-block): ctx rows [nq=128 x 4 j-tiles], cols 32 ctx + 1 den
    out_ctx = nc.dram_tensor(
        "out_ctx", [HPC * N_BLOCKS, 128, 4 * VTW], f32, kind="ExternalOutput"
    ).ap()

    tiles_plan, ob_eng = _exp_plan()

    with tile.TileContext(nc) as tc, ExitStack() as ctx:
        persist = ctx.enter_context(tc.tile_pool(name="persist", bufs=1))

        k8 = [persist.tile([48, 2 * N], fp8, name=f"k8s{h}") for h in range(HPC)]
        q8 = [persist.tile([48, 2 * N], fp8, name=f"q8s{h}") for h in range(HPC)]
        vt = [
            persist.tile([128, VTW * N_CHUNKS], bf16, name=f"vts{h}")
            for h in range(HPC)
        ]
        k8v = [k8[h].rearrange("p (k n) -> p k n", k=2) for h in range(HPC)]
        q8v = [q8[h].rearrange("p (k n) -> p k n", k=2) for h in range(HPC)]
        kqiv = [
            (t0.rearrange("p (k n) -> p k n", k=2), t1.rearrange("p (k n) -> p k n", k=2))
            for t0, t1 in kqi
        ]
        # load order: the first tiles of head 0 gate the pipeline start.
        # Spread the gating transfers across distinct DMA queues (SP/ACT/DVE
        # HWDGEs serialize per queue; the ~1.8us setup latency runs parallel).
        nc.sync.dma_start(out=q8v[0][:, :, 0:NQB], in_=kqiv[0][1][:, :, 0:NQB])
        nc.scalar.dma_start(out=k8v[0][:, :, 0:1024], in_=kqiv[0][0][:, :, 0:1024])
        nc.gpsimd.dma_start(out=vt[0], in_=vti[0])
        nc.sync.dma_start(out=k8v[0][:, :, 1024:N], in_=kqiv[0][0][:, :, 1024:N])
        nc.sync.dma_start(out=q8v[0][:, :, NQB:N], in_=kqiv[0][1][:, :, NQB:N])
        nc.sync.dma_start(out=k8[1], in_=kqi[1][0])
        nc.sync.dma_start(out=q8[1], in_=kqi[1][1])
        nc.sync.dma_start(out=vt[1], in_=vti[1])

        pools = {
            "A": ctx.enter_context(tc.tile_pool(name="scA", bufs=2, space="PSUM")),
            "D": ctx.enter_context(tc.tile_pool(name="scD", bufs=2, space="PSUM")),
        }
        ctx_pool = ctx.enter_context(tc.tile_pool(name="ctxp", bufs=2, space="PSUM"))
        ex_pool = ctx.enter_context(tc.tile_pool(name="ex", bufs=8))
        ob_pool = ctx.enter_context(tc.tile_pool(name="obp", bufs=4))

        # ---- attention ----
        for h in range(HPC):
            for b in range(N_BLOCKS):
                blk = h * N_BLOCKS + b
                ctx_ps = ctx_pool.tile([128, 4 * VTW], f32, name="ctx_ps")
                cbase = 0
                for eng, nch in tiles_plan[blk]:
                    w = nch * NQB
                    sc = pools[eng].tile([128, w], f32, name="sc" + eng, tag="sc" + eng)
                    with tc.high_priority(offset=320):
                        for u in range(nch):
                            c = cbase + u
                            nc.tensor.matmul(
                                out=sc[:, ts(u, NQB)],
                                lhsT=k8v[h][:, :, ts(c, NKC)],
                                rhs=q8v[h][:, :, ts(b, NQB)],
                                start=True,
                                stop=True,
                                perf_mode=PM.DoubleRow,
                            )
                    ex = ex_pool.tile([128, w], bf16, name="ex")
                    if eng == "A":
                        nc.scalar.activation(ex, sc, AF.Exp, scale=ACT_SCALE)
                    else:
                        nc.vector.tensor_scalar(
                            ex.bitcast(i16), sc, EXP2_A, EXP2_B,
                            op0=ALU.mult, op1=ALU.add,
                        )
                    for u in range(nch):
                        c = cbase + u
                        for j in range(4):
                            # NOTE: start=True clears has_written BANK-wide,
                            # so only the tile's very first matmul may set it
                            # (the bit-clear makes every region's first write
                            # an overwrite, later writes accumulate).
                            nc.tensor.matmul(
                                out=ctx_ps[:, ts(j, VTW)],
                                lhsT=ex[:, NQB * u + NKC * j : NQB * u + NKC * (j + 1)],
                                rhs=vt[h][:, ts(c, VTW)],
                                start=(c == 0 and j == 0),
                                stop=(c == N_CHUNKS - 1 and j == 3),
                                skip_group_check=True,
                            )
                    cbase += nch
                ob = ob_pool.tile([128, 4 * VTW], f32, name="ob")
                if ob_eng[blk] == "A":
                    nc.scalar.copy(out=ob, in_=ctx_ps)
                else:
                    nc.vector.tensor_copy(out=ob, in_=ctx_ps)
                nc.sync.dma_start(out=out_ctx[blk], in_=ob)

    nc.compile()
    return nc


def _shard_inputs(query, key, Wq, Wk, Wv):
    import ml_dtypes

    query = _f32(query).reshape(B, C, N)
    key = _f32(key).reshape(B, C, N)
    Wq, Wk, Wv = _f32(Wq), _f32(Wk), _f32(Wv)

    def pack96(G):
        # G [96, N] fp8-valued f32 -> [48, 2*N] with (p, j*N+n) = G[j*48+p, n]
        return np.ascontiguousarray(
            G.reshape(2, 48, N).transpose(1, 0, 2).reshape(48, 2 * N)
        )

    in_maps = []
    for core in range(NCORES):
        b, half = core // 2, core % 2
        im = {}
        for hl in range(HPC):
            ch0 = 64 * half + 32 * hl
            wq_h = Wq[ch0 : ch0 + 32, :]  # [32, 128]
            wk_h = Wk[ch0 : ch0 + 32, :]
            wv_h = Wv[ch0 : ch0 + 32, :]
            qs = BETA * (wq_h @ query[b])  # [32, N]
            ks = wk_h @ key[b]  # [32, N]
            q_hi = _fp8(qs)
            k_hi = _fp8(ks)
            q_lo = _fp8(qs - np.asarray(q_hi, np.float32))
            k_lo = _fp8(ks - np.asarray(k_hi, np.float32))
            # contraction rows: q_hi.k_hi + q_lo.k_hi + q_hi.k_lo
            Gq = np.concatenate([q_hi, q_lo, q_hi]).astype(ml_dtypes.float8_e4m3)
            Gk = np.concatenate([k_hi, k_hi, k_lo]).astype(ml_dtypes.float8_e4m3)
            im[f"q8_{hl}"] = pack96(Gq)
            im[f"k8_{hl}"] = pack96(Gk)
            vta = np.ones((N, VTW), np.float32)
            vta[:, :32] = key[b].T @ wv_h.T
            im[f"vt{hl}"] = _bf16(
                vta.reshape(N_CHUNKS, NKC, VTW)
                .transpose(1, 0, 2)
                .reshape(NKC, N_CHUNKS * VTW)
            )
        in_maps.append(im)
    return in_maps


def _run(in_maps, trace=False):
    from concourse import bass_utils

    nc = _build_program()
    return bass_utils.run_bass_kernel_spmd(
        nc, in_maps, core_ids=list(range(NCORES)), trace=trace
    )


def _assemble(results):
    out = np.empty((B, C, N), np.float32)
    for core in range(NCORES):
        b, half = core // 2, core % 2
        r = results[core]
        t = np.asarray(r["out_ctx"], np.float32)  # [16, 128, 132]
        t = t.reshape(HPC, N_BLOCKS, 128, 4, VTW)
        ctx = t[..., :32]  # [h, b8, p, j, d]
        den = t[..., 32]  # [h, b8, p, j]
        # nq index = b8*512 + j*128 + p -> order (b8, j, p)
        ctx = np.transpose(ctx, (0, 1, 3, 2, 4)).reshape(HPC, N, 32)
        den = np.transpose(den, (0, 1, 3, 2)).reshape(HPC, N)
        for hl in range(HPC):
            ch0 = 64 * half + 32 * hl
            out[b, ch0 : ch0 + 32, :] = (ctx[hl] / den[hl][:, None]).T
    return out.reshape(B, C, HS, WS)


def kernel(query, key, Wq, Wk, Wv):
    in_maps = _shard_inputs(query, key, Wq, Wk, Wv)
    res = _run(in_maps)
    return _assemble(res.results)


# revision 13
# speedup vs baseline: 1.1120x; 1.0475x over previous
"""Cross-attention kernel for Trainium2 (8 NeuronCores, SPMD data-parallel).

Problem: B=4, C=128, 64x64 spatial (N=4096 tokens), 4 heads of dim 32.
  q = Wq @ query; k = Wk @ key; v = Wv @ key   (1x1 convs == channel matmuls)
  out = softmax(q^T k / sqrt(32)) @ v          (per batch*head)

Sharding: 16 (batch, head) jobs -> 2 per core. Core i handles batch i//2,
heads {2*(i%2), 2*(i%2)+1} i.e. output channels [64*(i%2), 64*(i%2)+64).

Structure (per core):
  - Projections fold into host-side input prep (tiny vs. the N^2 attention).
    Per head the score matmul is rank-32: s = (Wq_h q)^T (Wk_h k). The fp8
    quantization error is cancelled to first order by a hi/lo split packed
    into the contraction dim (free under the PE cost model): 96 rows =
    {q_hi.k_hi, q_lo.k_hi, q_hi.k_lo} shipped as DoubleRow pairs [48, 2, N],
    so one fp8 DoubleRow matmul yields scoresT chunk [nk=128, nq=512] at
    half bf16 cost with ~bf16 accuracy.
  - exp: the N^2 score surface must drain PSUM->SBUF through ACT/DVE at
    1 elem/cycle/lane (GPSIMD and DMA cannot touch PSUM) - that drain is the
    kernel floor. Tiles are [128, 1024] (2 chunks, 2 PSUM banks) x 3 bufs so
    the exp(t) -> QK(t+3) -> exp(t+3) WAR chain amortizes over 3 tiles;
    engines are assigned by a greedy balance of modeled op costs
    (ACT ~1038ns vs DVE ~1192ns per tile).
  - PV flipped: ctx[nq=128, 33] += probsT_chunk(lhsT, stationary) @ v_aug
    (moving: 32 v columns + a ones column -> softmax denominator), bf16.
  - host: normalize by the denominator column and transpose to [C, N].
"""

import functools
import math

import numpy as np

NCORES = 8
B, C, HS, WS = 4, 128, 64, 64
N = HS * WS  # 4096 tokens
NUM_HEADS = 4
DH = 32  # head dim
HPC = 2  # heads per core

NQB = 512  # nq per QK matmul (one PSUM bank of f32)
NKC = 128  # nk chunk (PV contraction tile)
N_BLOCKS = N // NQB  # 8
N_CHUNKS = N // NKC  # 32
VTW = 33  # v^T tile width: 32 v cols + 1 ones col (denominator)
CPT = 2  # chunks per exp tile
TPB = N_CHUNKS // CPT  # 16 exp tiles per (h, block)

# host-side scale on the q projection (keeps fp8 operands ~unit variance)
BETA = 2.0
# device scores s_dev = BETA * (q.k); log2-domain y = s_dev * Y_PER_S
Y_PER_S = math.log2(math.e) / (BETA * math.sqrt(DH))
# Schraudolph exp2 in bf16: i16 = cvt(y*128 + (16256 - C)); bits = bf16 ~ 2^y
EXP2_A = 128.0 * Y_PER_S
EXP2_B = 16256.0 - 5.25
# ACT exact exp: exp(ACT_SCALE * s_dev) = 2^y
ACT_SCALE = math.log(2.0) * Y_PER_S

# modeled per-op engine costs (ns) used for the static plan
_COST_A_TILE = 1038.0  # ACT [128,1024] from PSUM
_COST_D_TILE = 658.0  # DVE [128,512] from PSUM
_COST_A_OB = 295.0
_COST_D_OB = 262.0
A_TILES = 9  # ACT tiles (2 chunks each) per block
D_TILES = N_CHUNKS - 2 * A_TILES  # DVE tiles (1 chunk each) per block
N_BLK = HPC * N_BLOCKS


def _exp_plan():
    """Per-block tile schedule: each engine owns a private double-buffered
    sc pool so its WAR turnaround hides under same-engine serial time.
    Interleave A (2-chunk) and D (1-chunk) tiles by modeled virtual finish
    time. The final block front-loads A tiles (and gets ob on ACT) so both
    engines drain together at the end. Returns (tiles, ob_eng):
    tiles[block] = list of (("A"|"D"), nchunks)."""
    tiles = []
    ob_eng = []
    ta = td = 0.0
    for blk in range(N_BLK):
        row = []
        na = nd = 0
        va = vd = 0.0
        while na < A_TILES or nd < D_TILES:
            pick_a = na < A_TILES and (
                nd >= D_TILES or va + _COST_A_TILE <= vd + _COST_D_TILE
            )
            if pick_a:
                row.append(("A", CPT))
                va += _COST_A_TILE
                na += 1
            else:
                row.append(("D", 1))
                vd += _COST_D_TILE
                nd += 1
        ta += va
        td += vd
        if blk == N_BLK - 1 or ta + _COST_A_OB <= td + _COST_D_OB:
            ob_eng.append("A")
            ta += _COST_A_OB
        else:
            ob_eng.append("D")
            td += _COST_D_OB
        tiles.append(row)
    return tiles, ob_eng


def _f32(x):
    return np.ascontiguousarray(np.asarray(x, dtype=np.float32))


def _bf16(x):
    import ml_dtypes

    return np.ascontiguousarray(
        np.asarray(x, dtype=np.float32).astype(ml_dtypes.bfloat16)
    )


def _fp8(x):
    import ml_dtypes

    return np.ascontiguousarray(
        np.asarray(x, dtype=np.float32).astype(ml_dtypes.float8_e4m3)
    )


@functools.lru_cache(maxsize=1)
def _build_program():
    from contextlib import ExitStack

    import concourse.tile as tile
    from concourse import bacc, mybir
    from concourse.bass import ts

    f32 = mybir.dt.float32
    bf16 = mybir.dt.bfloat16
    i16 = mybir.dt.int16
    fp8 = mybir.dt.float8e4
    AF = mybir.ActivationFunctionType
    ALU = mybir.AluOpType
    PM = mybir.MatmulPerfMode

    nc = bacc.Bacc(
        "TRN2",
        target_bir_lowering=False,
        debug=False,
        enable_asserts=False,
        num_devices=NCORES,
    )

    # hi/lo fp8 pair layout [48, 2*N]: element (p, j*N + n) = G[j*48 + p, n]
    # where G is the 96-row stack (see _shard_inputs).
    kqi = [
        (
            nc.dram_tensor(f"k8_{h}", [48, 2 * N], fp8, kind="ExternalInput").ap(),
            nc.dram_tensor(f"q8_{h}", [48, 2 * N], fp8, kind="ExternalInput").ap(),
        )
        for h in range(HPC)
    ]
    vti = [
        nc.dram_tensor(f"vt{h}", [128, VTW * N_CHUNKS], bf16, kind="ExternalInput").ap()
        for h in range(HPC)
    ]

    # per (h, nq-block): ctx rows [nq=128 x 4 j-tiles], cols 32 ctx + 1 den
    out_ctx = nc.dram_tensor(
        "out_ctx", [HPC * N_BLOCKS, 128, 4 * VTW], f32, kind="ExternalOutput"
    ).ap()

    tiles_plan, ob_eng = _exp_plan()

    with tile.TileContext(nc) as tc, ExitStack() as ctx:
        persist = ctx.enter_context(tc.tile_pool(name="persist", bufs=1))

        k8 = [persist.tile([48, 2 * N], fp8, name=f"k8s{h}") for h in range(HPC)]
        q8 = [persist.tile([48, 2 * N], fp8, name=f"q8s{h}") for h in range(HPC)]
        vt = [
            persist.tile([128, VTW * N_CHUNKS], bf16, name=f"vts{h}")
            for h in range(HPC)
        ]
        k8v = [k8[h].rearrange("p (k n) -> p k n", k=2) for h in range(HPC)]
        q8v = [q8[h].rearrange("p (k n) -> p k n", k=2) for h in range(HPC)]
        kqiv = [
            (t0.rearrange("p (k n) -> p k n", k=2), t1.rearrange("p (k n) -> p k n", k=2))
            for t0, t1 in kqi
        ]
        # load order: the first tiles of head 0 gate the pipeline start.
        # Spread the gating transfers across distinct DMA queues (SP/ACT/DVE
        # HWDGEs serialize per queue; the ~1.8us setup latency runs parallel).
        nc.sync.dma_start(out=q8v[0][:, :, 0:NQB], in_=kqiv[0][1][:, :, 0:NQB])
        nc.scalar.dma_start(out=k8v[0][:, :, 0:1024], in_=kqiv[0][0][:, :, 0:1024])
        nc.gpsimd.dma_start(out=vt[0], in_=vti[0])
        nc.sync.dma_start(out=k8v[0][:, :, 1024:N], in_=kqiv[0][0][:, :, 1024:N])
        nc.sync.dma_start(out=q8v[0][:, :, NQB:N], in_=kqiv[0][1][:, :, NQB:N])
        nc.sync.dma_start(out=k8[1], in_=kqi[1][0])
        nc.sync.dma_start(out=q8[1], in_=kqi[1][1])
        nc.sync.dma_start(out=vt[1], in_=vti[1])

        pools = {
            "A": ctx.enter_context(tc.tile_pool(name="scA", bufs=2, space="PSUM")),
            "D": ctx.enter_context(tc.tile_pool(name="scD", bufs=2, space="PSUM")),
        }
        ctx_pool = ctx.enter_context(tc.tile_pool(name="ctxp", bufs=2, space="PSUM"))
        ex_pool = ctx.enter_context(tc.tile_pool(name="ex", bufs=8))
        ob_pool = ctx.enter_context(tc.tile_pool(name="obp", bufs=4))

        # ---- attention ----
        for h in range(HPC):
            for b in range(N_BLOCKS):
                blk = h * N_BLOCKS + b
                ctx_ps = ctx_pool.tile([128, 4 * VTW], f32, name="ctx_ps")
                cbase = 0
                for eng, nch in tiles_plan[blk]:
                    w = nch * NQB
                    sc = pools[eng].tile([128, w], f32, name="sc" + eng, tag="sc" + eng)
                    with tc.high_priority(offset=320):
                        for u in range(nch):
                            c = cbase + u
                            nc.tensor.matmul(
                                out=sc[:, ts(u, NQB)],
                                lhsT=k8v[h][:, :, ts(c, NKC)],
                                rhs=q8v[h][:, :, ts(b, NQB)],
                                start=True,
                                stop=True,
                                perf_mode=PM.DoubleRow,
                            )
                    ex = ex_pool.tile([128, w], bf16, name="ex")
                    if eng == "A":
                        nc.scalar.activation(ex, sc, AF.Exp, scale=ACT_SCALE)
                    else:
                        nc.vector.tensor_scalar(
                            ex.bitcast(i16), sc, EXP2_A, EXP2_B,
                            op0=ALU.mult, op1=ALU.add,
                        )
                    for u in range(nch):
                        c = cbase + u
                        for j in range(4):
                            # NOTE: start=True clears has_written BANK-wide,
                            # so only the tile's very first matmul may set it
                            # (the bit-clear makes every region's first write
                            # an overwrite, later writes accumulate).
                            nc.tensor.matmul(
                                out=ctx_ps[:, ts(j, VTW)],
                                lhsT=ex[:, NQB * u + NKC * j : NQB * u + NKC * (j + 1)],
                                rhs=vt[h][:, ts(c, VTW)],
                                start=(c == 0 and j == 0),
                                stop=(c == N_CHUNKS - 1 and j == 3),
                                skip_group_check=True,
                            )
                    cbase += nch
                ob = ob_pool.tile([128, 4 * VTW], f32, name="ob")
                if ob_eng[blk] == "A":
                    nc.scalar.copy(out=ob, in_=ctx_ps)
                else:
                    nc.vector.tensor_copy(out=ob, in_=ctx_ps)
                nc.sync.dma_start(out=out_ctx[blk], in_=ob)

    nc.compile()
    return nc


def _shard_inputs(query, key, Wq, Wk, Wv):
    import ml_dtypes

    query = _f32(query).reshape(B, C, N)
    key = _f32(key).reshape(B, C, N)
    Wq, Wk, Wv = _f32(Wq), _f32(Wk), _f32(Wv)

    def pack96(G):
        # G [96, N] fp8-valued f32 -> [48, 2*N] with (p, j*N+n) = G[j*48+p, n]
        return np.ascontiguousarray(
            G.reshape(2, 48, N).transpose(1, 0, 2).reshape(48, 2 * N)
        )

    in_maps = []
    for core in range(NCORES):
        b, half = core // 2, core % 2
        im = {}
        for hl in range(HPC):
            ch0 = 64 * half + 32 * hl
            wq_h = Wq[ch0 : ch0 + 32, :]  # [32, 128]
            wk_h = Wk[ch0 : ch0 + 32, :]
            wv_h = Wv[ch0 : ch0 + 32, :]
            qs = BETA * (wq_h @ query[b])  # [32, N]
            ks = wk_h @ key[b]  # [32, N]
            q_hi = _fp8(qs)
            k_hi = _fp8(ks)
            q_lo = _fp8(qs - np.asarray(q_hi, np.float32))
            k_lo = _fp8(ks - np.asarray(k_hi, np.float32))
            # contraction rows: q_hi.k_hi + q_lo.k_hi + q_hi.k_lo
            Gq = np.concatenate([q_hi, q_lo, q_hi]).astype(ml_dtypes.float8_e4m3)
            Gk = np.concatenate([k_hi, k_hi, k_lo]).astype(ml_dtypes.float8_e4m3)
            im[f"q8_{hl}"] = pack96(Gq)
            im[f"k8_{hl}"] = pack96(Gk)
            vta = np.ones((N, VTW), np.float32)
            vta[:, :32] = key[b].T @ wv_h.T
            im[f"vt{hl}"] = _bf16(
                vta.reshape(N_CHUNKS, NKC, VTW)
                .transpose(1, 0, 2)
                .reshape(NKC, N_CHUNKS * VTW)
            )
        in_maps.append(im)
    return in_maps


def _run(in_maps, trace=False):
    from concourse import bass_utils

    nc = _build_program()
    return bass_utils.run_bass_kernel_spmd(
        nc, in_maps, core_ids=list(range(NCORES)), trace=trace
    )


def _assemble(results):
    out = np.empty((B, C, N), np.float32)
    for core in range(NCORES):
        b, half = core // 2, core % 2
        r = results[core]
        t = np.asarray(r["out_ctx"], np.float32)  # [16, 128, 132]
        t = t.reshape(HPC, N_BLOCKS, 128, 4, VTW)
        ctx = t[..., :32]  # [h, b8, p, j, d]
        den = t[..., 32]  # [h, b8, p, j]
        # nq index = b8*512 + j*128 + p -> order (b8, j, p)
        ctx = np.transpose(ctx, (0, 1, 3, 2, 4)).reshape(HPC, N, 32)
        den = np.transpose(den, (0, 1, 3, 2)).reshape(HPC, N)
        for hl in range(HPC):
            ch0 = 64 * half + 32 * hl
            out[b, ch0 : ch0 + 32, :] = (ctx[hl] / den[hl][:, None]).T
    return out.reshape(B, C, HS, WS)


def kernel(query, key, Wq, Wk, Wv):
    in_maps = _shard_inputs(query, key, Wq, Wk, Wv)
    res = _run(in_maps)
    return _assemble(res.results)


# revision 14
# speedup vs baseline: 1.1184x; 1.0058x over previous
"""Cross-attention kernel for Trainium2 (8 NeuronCores, SPMD data-parallel).

Problem: B=4, C=128, 64x64 spatial (N=4096 tokens), 4 heads of dim 32.
  q = Wq @ query; k = Wk @ key; v = Wv @ key   (1x1 convs == channel matmuls)
  out = softmax(q^T k / sqrt(32)) @ v          (per batch*head)

Sharding: 16 (batch, head) jobs -> 2 per core. Core i handles batch i//2,
heads {2*(i%2), 2*(i%2)+1} i.e. output channels [64*(i%2), 64*(i%2)+64).

Structure (per core):
  - Projections fold into host-side input prep (tiny vs. the N^2 attention).
    Per head the score matmul is rank-32: s = (Wq_h q)^T (Wk_h k). The fp8
    quantization error is cancelled to first order by a hi/lo split packed
    into the contraction dim (free under the PE cost model): 96 rows =
    {q_hi.k_hi, q_lo.k_hi, q_hi.k_lo} shipped as DoubleRow pairs [48, 2, N],
    so one fp8 DoubleRow matmul yields scoresT chunk [nk=128, nq=512] at
    half bf16 cost with ~bf16 accuracy.
  - exp: the N^2 score surface must drain PSUM->SBUF through ACT/DVE at
    1 elem/cycle/lane (GPSIMD and DMA cannot touch PSUM) - that drain is the
    kernel floor. Tiles are [128, 1024] (2 chunks, 2 PSUM banks) x 3 bufs so
    the exp(t) -> QK(t+3) -> exp(t+3) WAR chain amortizes over 3 tiles;
    engines are assigned by a greedy balance of modeled op costs
    (ACT ~1038ns vs DVE ~1192ns per tile).
  - PV flipped: ctx[nq=128, 33] += probsT_chunk(lhsT, stationary) @ v_aug
    (moving: 32 v columns + a ones column -> softmax denominator), bf16.
  - host: normalize by the denominator column and transpose to [C, N].
"""

import functools
import math

import numpy as np

NCORES = 8
B, C, HS, WS = 4, 128, 64, 64
N = HS * WS  # 4096 tokens
NUM_HEADS = 4
DH = 32  # head dim
HPC = 2  # heads per core

NQB = 512  # nq per QK matmul (one PSUM bank of f32)
NKC = 128  # nk chunk (PV contraction tile)
N_BLOCKS = N // NQB  # 8
N_CHUNKS = N // NKC  # 32
VTW = 33  # v^T tile width: 32 v cols + 1 ones col (denominator)
CPT = 2  # chunks per exp tile
TPB = N_CHUNKS // CPT  # 16 exp tiles per (h, block)

# host-side scale on the q projection (keeps fp8 operands ~unit variance)
BETA = 2.0
# device scores s_dev = BETA * (q.k); log2-domain y = s_dev * Y_PER_S
Y_PER_S = math.log2(math.e) / (BETA * math.sqrt(DH))
# Schraudolph exp2 in bf16: i16 = cvt(y*128 + (16256 - C)); bits = bf16 ~ 2^y
EXP2_A = 128.0 * Y_PER_S
EXP2_B = 16256.0 - 5.25
# ACT exact exp: exp(ACT_SCALE * s_dev) = 2^y
ACT_SCALE = math.log(2.0) * Y_PER_S

# modeled per-op engine costs (ns) used for the static plan
_COST_A_TILE = 1038.0  # ACT [128,1024] from PSUM
_COST_D_TILE = 658.0  # DVE [128,512] from PSUM
_COST_A_OB = 295.0
_COST_D_OB = 262.0
A_TILES = 9  # ACT tiles (2 chunks each) per block
D_TILES = N_CHUNKS - 2 * A_TILES  # DVE tiles (1 chunk each) per block
N_BLK = HPC * N_BLOCKS


def _exp_plan():
    """Per-block tile schedule: each engine owns a private double-buffered
    sc pool so its WAR turnaround hides under same-engine serial time.
    Interleave A (2-chunk) and D (1-chunk) tiles by modeled virtual finish
    time. The final block front-loads A tiles (and gets ob on ACT) so both
    engines drain together at the end. Returns (tiles, ob_eng):
    tiles[block] = list of (("A"|"D"), nchunks)."""
    tiles = []
    ob_eng = []
    ta = td = 0.0
    for blk in range(N_BLK):
        row = []
        na = nd = 0
        va = vd = 0.0
        while na < A_TILES or nd < D_TILES:
            pick_a = na < A_TILES and (
                nd >= D_TILES or va + _COST_A_TILE <= vd + _COST_D_TILE
            )
            if pick_a:
                row.append(("A", CPT))
                va += _COST_A_TILE
                na += 1
            else:
                row.append(("D", 1))
                vd += _COST_D_TILE
                nd += 1
        ta += va
        td += vd
        if ta + _COST_A_OB <= td + _COST_D_OB:
            ob_eng.append("A")
            ta += _COST_A_OB
        else:
            ob_eng.append("D")
            td += _COST_D_OB
        tiles.append(row)
    return tiles, ob_eng


def _f32(x):
    return np.ascontiguousarray(np.asarray(x, dtype=np.float32))


def _bf16(x):
    import ml_dtypes

    return np.ascontiguousarray(
        np.asarray(x, dtype=np.float32).astype(ml_dtypes.bfloat16)
    )


def _fp8(x):
    import ml_dtypes

    return np.ascontiguousarray(
        np.asarray(x, dtype=np.float32).astype(ml_dtypes.float8_e4m3)
    )


@functools.lru_cache(maxsize=1)
def _build_program():
    from contextlib import ExitStack

    import concourse.tile as tile
    from concourse import bacc, mybir
    from concourse.bass import ts

    f32 = mybir.dt.float32
    bf16 = mybir.dt.bfloat16
    i16 = mybir.dt.int16
    fp8 = mybir.dt.float8e4
    AF = mybir.ActivationFunctionType
    ALU = mybir.AluOpType
    PM = mybir.MatmulPerfMode

    nc = bacc.Bacc(
        "TRN2",
        target_bir_lowering=False,
        debug=False,
        enable_asserts=False,
        num_devices=NCORES,
    )

    # hi/lo fp8 pair layout [48, 2*N]: element (p, j*N + n) = G[j*48 + p, n]
    # where G is the 96-row stack (see _shard_inputs).
    kqi = [
        (
            nc.dram_tensor(f"k8_{h}", [48, 2 * N], fp8, kind="ExternalInput").ap(),
            nc.dram_tensor(f"q8_{h}", [48, 2 * N], fp8, kind="ExternalInput").ap(),
        )
        for h in range(HPC)
    ]
    vti = [
        nc.dram_tensor(f"vt{h}", [128, VTW * N_CHUNKS], bf16, kind="ExternalInput").ap()
        for h in range(HPC)
    ]

    # per (h, nq-block): ctx rows [nq=128 x 4 j-tiles], cols 32 ctx + 1 den
    out_ctx = nc.dram_tensor(
        "out_ctx", [HPC * N_BLOCKS, 128, 4 * VTW], f32, kind="ExternalOutput"
    ).ap()

    tiles_plan, ob_eng = _exp_plan()

    with tile.TileContext(nc) as tc, ExitStack() as ctx:
        persist = ctx.enter_context(tc.tile_pool(name="persist", bufs=1))

        k8 = [persist.tile([48, 2 * N], fp8, name=f"k8s{h}") for h in range(HPC)]
        q8 = [persist.tile([48, 2 * N], fp8, name=f"q8s{h}") for h in range(HPC)]
        vt = [
            persist.tile([128, VTW * N_CHUNKS], bf16, name=f"vts{h}")
            for h in range(HPC)
        ]
        k8v = [k8[h].rearrange("p (k n) -> p k n", k=2) for h in range(HPC)]
        q8v = [q8[h].rearrange("p (k n) -> p k n", k=2) for h in range(HPC)]
        kqiv = [
            (t0.rearrange("p (k n) -> p k n", k=2), t1.rearrange("p (k n) -> p k n", k=2))
            for t0, t1 in kqi
        ]
        # load order: the first tiles of head 0 gate the pipeline start.
        # Spread the gating transfers across distinct DMA queues (SP/ACT/DVE
        # HWDGEs serialize per queue; the ~1.8us setup latency runs parallel).
        nc.sync.dma_start(out=q8v[0][:, :, 0:NQB], in_=kqiv[0][1][:, :, 0:NQB])
        nc.sync.dma_start(out=k8v[0][:, :, 0:1024], in_=kqiv[0][0][:, :, 0:1024])
        nc.sync.dma_start(out=vt[0], in_=vti[0])
        nc.sync.dma_start(out=k8v[0][:, :, 1024:N], in_=kqiv[0][0][:, :, 1024:N])
        nc.sync.dma_start(out=q8v[0][:, :, NQB:N], in_=kqiv[0][1][:, :, NQB:N])
        nc.sync.dma_start(out=k8[1], in_=kqi[1][0])
        nc.sync.dma_start(out=q8[1], in_=kqi[1][1])
        nc.sync.dma_start(out=vt[1], in_=vti[1])

        pools = {
            "A": ctx.enter_context(tc.tile_pool(name="scA", bufs=2, space="PSUM")),
            "D": ctx.enter_context(tc.tile_pool(name="scD", bufs=2, space="PSUM")),
        }
        ctx_pool = ctx.enter_context(tc.tile_pool(name="ctxp", bufs=2, space="PSUM"))
        ex_pool = ctx.enter_context(tc.tile_pool(name="ex", bufs=8))
        ob_pool = ctx.enter_context(tc.tile_pool(name="obp", bufs=4))

        # ---- attention ----
        for h in range(HPC):
            for b in range(N_BLOCKS):
                blk = h * N_BLOCKS + b
                ctx_ps = ctx_pool.tile([128, 4 * VTW], f32, name="ctx_ps")
                cbase = 0
                for eng, nch in tiles_plan[blk]:
                    w = nch * NQB
                    sc = pools[eng].tile([128, w], f32, name="sc" + eng, tag="sc" + eng)
                    with tc.high_priority(offset=320):
                        for u in range(nch):
                            c = cbase + u
                            nc.tensor.matmul(
                                out=sc[:, ts(u, NQB)],
                                lhsT=k8v[h][:, :, ts(c, NKC)],
                                rhs=q8v[h][:, :, ts(b, NQB)],
                                start=True,
                                stop=True,
                                perf_mode=PM.DoubleRow,
                            )
                    ex = ex_pool.tile([128, w], bf16, name="ex")
                    if eng == "A":
                        nc.scalar.activation(ex, sc, AF.Exp, scale=ACT_SCALE)
                    else:
                        nc.vector.tensor_scalar(
                            ex.bitcast(i16), sc, EXP2_A, EXP2_B,
                            op0=ALU.mult, op1=ALU.add,
                        )
                    for u in range(nch):
                        c = cbase + u
                        for j in range(4):
                            # NOTE: start=True clears has_written BANK-wide,
                            # so only the tile's very first matmul may set it
                            # (the bit-clear makes every region's first write
                            # an overwrite, later writes accumulate).
                            nc.tensor.matmul(
                                out=ctx_ps[:, ts(j, VTW)],
                                lhsT=ex[:, NQB * u + NKC * j : NQB * u + NKC * (j + 1)],
                                rhs=vt[h][:, ts(c, VTW)],
                                start=(c == 0 and j == 0),
                                stop=(c == N_CHUNKS - 1 and j == 3),
                                skip_group_check=True,
                            )
                    cbase += nch
                ob = ob_pool.tile([128, 4 * VTW], f32, name="ob")
                if ob_eng[blk] == "A":
                    nc.scalar.copy(out=ob, in_=ctx_ps)
                else:
                    nc.vector.tensor_copy(out=ob, in_=ctx_ps)
                nc.sync.dma_start(out=out_ctx[blk], in_=ob)

    nc.compile()
    return nc


def _shard_inputs(query, key, Wq, Wk, Wv):
    import ml_dtypes

    query = _f32(query).reshape(B, C, N)
    key = _f32(key).reshape(B, C, N)
    Wq, Wk, Wv = _f32(Wq), _f32(Wk), _f32(Wv)

    def pack96(G):
        # G [96, N] fp8-valued f32 -> [48, 2*N] with (p, j*N+n) = G[j*48+p, n]
        return np.ascontiguousarray(
            G.reshape(2, 48, N).transpose(1, 0, 2).reshape(48, 2 * N)
        )

    in_maps = []
    for core in range(NCORES):
        b, half = core // 2, core % 2
        im = {}
        for hl in range(HPC):
            ch0 = 64 * half + 32 * hl
            wq_h = Wq[ch0 : ch0 + 32, :]  # [32, 128]
            wk_h = Wk[ch0 : ch0 + 32, :]
            wv_h = Wv[ch0 : ch0 + 32, :]
            qs = BETA * (wq_h @ query[b])  # [32, N]
            ks = wk_h @ key[b]  # [32, N]
            q_hi = _fp8(qs)
            k_hi = _fp8(ks)
            q_lo = _fp8(qs - np.asarray(q_hi, np.float32))
            k_lo = _fp8(ks - np.asarray(k_hi, np.float32))
            # contraction rows: q_hi.k_hi + q_lo.k_hi + q_hi.k_lo
            Gq = np.concatenate([q_hi, q_lo, q_hi]).astype(ml_dtypes.float8_e4m3)
            Gk = np.concatenate([k_hi, k_hi, k_lo]).astype(ml_dtypes.float8_e4m3)
            im[f"q8_{hl}"] = pack96(Gq)
            im[f"k8_{hl}"] = pack96(Gk)
            vta = np.ones((N, VTW), np.float32)
            vta[:, :32] = key[b].T @ wv_h.T
            im[f"vt{hl}"] = _bf16(
                vta.reshape(N_CHUNKS, NKC, VTW)
                .transpose(1, 0, 2)
                .reshape(NKC, N_CHUNKS * VTW)
            )
        in_maps.append(im)
    return in_maps


def _run(in_maps, trace=False):
    from concourse import bass_utils

    nc = _build_program()
    return bass_utils.run_bass_kernel_spmd(
        nc, in_maps, core_ids=list(range(NCORES)), trace=trace
    )


def _assemble(results):
    out = np.empty((B, C, N), np.float32)
    for core in range(NCORES):
        b, half = core // 2, core % 2
        r = results[core]
        t = np.asarray(r["out_ctx"], np.float32)  # [16, 128, 132]
        t = t.reshape(HPC, N_BLOCKS, 128, 4, VTW)
        ctx = t[..., :32]  # [h, b8, p, j, d]
        den = t[..., 32]  # [h, b8, p, j]
        # nq index = b8*512 + j*128 + p -> order (b8, j, p)
        ctx = np.transpose(ctx, (0, 1, 3, 2, 4)).reshape(HPC, N, 32)
        den = np.transpose(den, (0, 1, 3, 2)).reshape(HPC, N)
        for hl in range(HPC):
            ch0 = 64 * half + 32 * hl
            out[b, ch0 : ch0 + 32, :] = (ctx[hl] / den[hl][:, None]).T
    return out.reshape(B, C, HS, WS)


def kernel(query, key, Wq, Wk, Wv):
    in_maps = _shard_inputs(query, key, Wq, Wk, Wv)
    res = _run(in_maps)
    return _assemble(res.results)
